# revision 22
# baseline (speedup 1.0000x reference)
"""Trainium2 Bass kernel for nn_BigramLanguageModel (6-layer dense transformer).

Sharding: DP-2 over batch (one die of 4 cores per batch element) x TP-4
Megatron over heads / FFN hidden / vocab within each group.

Layout convention: activations are feature-major ("xT" = [D, T]) and stored
in SBUF as [128, D//128, T] (feature d -> partition d%128, subtile d//128).
All matmuls run in bf16 with fp32 PSUM accumulation. The residual stream
stays fp32 in SBUF.

Attention per (batch, head), with scores transposed (S^T = [s, t]) so the
causal mask is block-triangular and only diagonal 128x128 blocks need
masking. Softmax uses the expm1 decomposition to survive bf16:
  exp(s) = 1 + E' with E' = exp(s) - 1; the scores here are ~1e-3 so E'
  carries the signal at full relative precision in bf16, while the "1"
  part is summed exactly via a triangular-ones matmul plus a running
  column-sum prefix of V_ext = [ones | V] (the ones column puts the
  softmax normalizer on PSUM partition 0). Normalization is delayed past
  the A@V matmul and applied per token via reciprocal + partition
  broadcast, fused into the PSUM eviction.

Per layer cross-core comms: AllGather of the per-rank attention head
outputs (each core then applies the full Wo), and an AllReduce of the
row-split FFN W2 partial outputs. Both in bf16.
"""

import numpy as np
import ml_dtypes

import concourse.bass as bass
import concourse.mybir as mybir
import concourse.tile as tile
from concourse import bacc
from concourse.bass_utils import run_bass_kernel_spmd

FP32 = mybir.dt.float32
BF16 = mybir.dt.bfloat16
AX = mybir.AluOpType
FP32R = mybir.dt.float32r

# Model constants
B, D, H, HD, V = 2, 1024, 16, 64, 32000
KS = D // 128          # 8 feature subtiles
TP = 4                 # tensor-parallel group size
HLOC = H // TP         # 4 heads per core
FLOC = 4 * D // TP     # 1024 ffn hidden per core
VPAD = 32768
VLOC = VPAD // TP      # 8192 vocab columns per core
EPS = 1e-5
SCALE = float(D) ** -0.5

GROUPS = [[0, 1, 2, 3], [4, 5, 6, 7]]

# debug/timing switches (ab_test.py)
SKIP_COLLECTIVES = False
SKIP_LOGITS = False


def build_nc(T=1024, L=6):
    """Build the SPMD program (identical on all 8 cores)."""
    TB = T // 128                      # t/s 128-blocks
    THW = min(512, T)                  # matmul N chunk
    NTH = T // THW                     # n chunks over t
    VT = VLOC // 128                   # 64 vocab 128-tiles
    VCH = 4                            # vocab tiles per wf DMA chunk
    assert T % 128 == 0 and THW % 128 == 0

    nc = bacc.Bacc(None, target_bir_lowering=False, debug=False, num_devices=8)

    # ---- DRAM parameters (per-core shards prepared on host) ----
    d_x0 = nc.declare_dram_parameter("x0", [128, KS, T], FP32R, isOutput=False)
    d_wqk = nc.declare_dram_parameter("wqk", [128, KS, L, 2, 2, 128], FP32R, isOutput=False)
    d_wv = nc.declare_dram_parameter("wv", [128, KS, L, 256], FP32R, isOutput=False)
    d_wo = nc.declare_dram_parameter("wo", [128, KS, L, 1024], BF16, isOutput=False)
    d_w1 = nc.declare_dram_parameter("w1", [128, KS, L, 1024], FP32R, isOutput=False)
    d_w2 = nc.declare_dram_parameter("w2", [128, KS, L, 1024], BF16, isOutput=False)
    d_b1 = nc.declare_dram_parameter("b1s", [128, KS * L], FP32, isOutput=False)
    d_bo2 = nc.declare_dram_parameter("bo2", [128, KS * L * 2], FP32, isOutput=False)
    d_wf = nc.declare_dram_parameter("wf", [128, KS, VLOC], BF16, isOutput=False)
    d_bfb = nc.declare_dram_parameter("bfb", [128, VT], FP32, isOutput=False)
    d_lng = nc.declare_dram_parameter("lng", [128, KS], FP32, isOutput=False)
    d_lnb = nc.declare_dram_parameter("lnb", [128, KS], FP32, isOutput=False)
    d_tri = nc.declare_dram_parameter("tri", [128, 128], FP32, isOutput=False)
    d_out = nc.declare_dram_parameter("lg", [128, VT, T], FP32, isOutput=True)

    with tile.TileContext(nc) as tc:
        with (
            tc.tile_pool(name="persist", bufs=1) as pp,
            tc.tile_pool(name="big", bufs=2) as bigp,
            tc.tile_pool(name="ps512", bufs=4, space="PSUM") as ps512,
            tc.tile_pool(name="ps_o", bufs=1, space="PSUM") as pso,
            tc.tile_pool(name="ps_cs", bufs=1, space="PSUM") as pscs,
            tc.tile_pool(name="ps_mu", bufs=1, space="PSUM") as psmu,
            tc.tile_pool(name="dram", bufs=1, space="DRAM") as dp,
        ):
            # ---- persistent SBUF state ----
            x_sb = pp.tile([128, KS, T], FP32R, name="x_sb")
            qk_sb = pp.tile([128, 4, T], BF16, name="qk_sb")  # q pairs 0-1, k pairs 2-3
            vsb = pp.tile([128, TB, HLOC, 65], BF16, name="vsb")  # col 64 = ones
            osb = pp.tile([64, HLOC, T], BF16, name="osb")
            r_row = pp.tile([1, T], FP32, name="r_row")
            mu_row = pp.tile([1, T], FP32, name="mu_row")
            bc = pp.tile([128, T], FP32, name="bc")
            # packed f32 scalar/constant columns
            C_B1 = 208
            C_BO2 = C_B1 + KS * L
            C_EPS = C_BO2 + KS * 2 * L
            C_C = C_EPS + 1
            smf = pp.tile([128, C_C + 2 * (TB + 2)], FP32, name="smf")
            # packed bf16: 0:128 tri_b | 128:129 ones_col
            smb = pp.tile([128, 132], BF16, name="smb")

            tri_f = smf[:, 80:208]
            tri_b = smb[:, 0:128]
            ones_col = smb[:, 128:129]

            nc.sync.dma_start(x_sb[:], d_x0[:])
            nc.sync.dma_start(smf[:, 80:208], d_tri[:])
            nc.sync.dma_start(smf[:, C_B1:C_B1 + KS * L], d_b1[:])
            nc.sync.dma_start(smf[:, C_BO2:C_BO2 + KS * 2 * L], d_bo2[:])
            nc.sync.dma_start(smf[:, 0:8], d_lng[:])
            nc.sync.dma_start(smf[:, 8:16], d_lnb[:])
            nc.sync.dma_start(smf[:, 16:80], d_bfb[:])
            nc.vector.tensor_copy(tri_b, tri_f)
            nc.any.memset(ones_col, 1.0)
            nc.any.memset(smf[0:1, C_EPS:C_EPS + 1], EPS)
            nc.any.memset(vsb[:, :, :, 64:65], 1.0)

            def xr(k, tsl):
                return x_sb[:, k, tsl]

            def b1_ap(m, l):
                c0 = C_B1 + m * L + l
                return smf[:, c0:c0 + 1]

            def bo2_ap(m, l, which):
                c0 = C_BO2 + m * 2 * L + l * 2 + which
                return smf[:, c0:c0 + 1]

            with (
                tc.tile_pool(name="wq_pool", bufs=1) as wqp,
                tc.tile_pool(name="wo_pool", bufs=1) as wop,
                tc.tile_pool(name="w1_pool", bufs=1) as w1p,
                tc.tile_pool(name="w2_pool", bufs=1) as w2p,
                tc.tile_pool(name="etmp_pool", bufs=1) as etp,
            ):
                for l in range(L):
                    # ---- layer weight loads (slot reuse staggers these) ----
                    wqk = wqp.tile([128, KS, 2, 2, 128], FP32R, tag="wqk")
                    wv = wqp.tile([128, KS, 256], FP32R, tag="wv")
                    wo = wop.tile([128, KS, 1024], BF16, tag="wo")
                    w1 = w1p.tile([128, KS, 1024], FP32R, tag="w1")
                    w2 = w2p.tile([128, KS, 1024], BF16, tag="w2")
                    nc.sync.dma_start(wqk[:], d_wqk[:, :, l])
                    nc.sync.dma_start(wv[:], d_wv[:, :, l])
                    nc.sync.dma_start(wo[:], d_wo[:, :, l])
                    nc.sync.dma_start(w1[:], d_w1[:, :, l])
                    nc.sync.dma_start(w2[:], d_w2[:, :, l])

                    # ---- QKV projections (t-half major) ----
                    for th in range(NTH):
                        tsl = slice(th * THW, (th + 1) * THW)
                        for pair in range(2):
                            for qk in range(2):
                                ps = ps512.tile([128, THW], FP32, tag="ps512")
                                for k in range(KS):
                                    nc.tensor.matmul(
                                        ps[:],
                                        wqk[:, k, pair, qk, :],
                                        xr(k, tsl),
                                        start=(k == 0), stop=(k == KS - 1))
                                nc.vector.tensor_copy(
                                    qk_sb[:, 2 * qk + pair, tsl], ps[:])
                        for tb in range(th * (THW // 128),
                                        (th + 1) * (THW // 128)):
                            ps = ps512.tile([128, THW], FP32, tag="ps512")
                            for k in range(KS):
                                nc.tensor.matmul(
                                    ps[:, 0:256],
                                    xr(k, slice(tb * 128, (tb + 1) * 128)),
                                    wv[:, k, :],
                                    start=(k == 0), stop=(k == KS - 1))
                            nc.vector.tensor_copy(
                                vsb[:, tb, :, 0:64],
                                ps[:, 0:256].rearrange("p (h e) -> p h e", e=64))

                    # ---- attention per local head ----
                    ag_ins = [dp.tile([64, HLOC, THW], BF16, tag=f"ag_in{th}",
                                      name=f"ag_in{th}") for th in range(NTH)]
                    ag_outs = [dp.tile([TP, 64, HLOC, THW], BF16,
                                       tag=f"ag_out{th}", name=f"ag_out{th}")
                               for th in range(NTH)]
                    for h in range(HLOC):
                        hp = 64 * (h % 2)
                        hs = h // 2
                        esb = bigp.tile([128, TB, T], BF16, tag="big")
                        # scores S^T[s,t] -> E' = exp(s*SCALE)-1 (masked)
                        for i in range(TB):
                            for th in range(NTH):
                                t0 = th * THW
                                if t0 + THW <= 128 * i:
                                    continue  # fully below diagonal: unused
                                sc = ps512.tile([128, THW], FP32, tag="ps512")
                                nc.tensor.matmul(
                                    sc[:],
                                    qk_sb[hp:hp + 64, 2 + hs,
                                          i * 128:(i + 1) * 128],
                                    qk_sb[hp:hp + 64, hs, t0:t0 + THW],
                                    start=True, stop=True)
                                et = etp.tile([128, THW], FP32, tag="etmp")
                                nc.scalar.activation(
                                    et[:], sc[:],
                                    mybir.ActivationFunctionType.Exp,
                                    scale=SCALE)
                                # diagonal 128-block inside this (i, th)?
                                dlo = i * 128 - t0
                                if 0 <= dlo < THW:
                                    nc.vector.scalar_tensor_tensor(
                                        esb[:, i, i * 128:(i + 1) * 128],
                                        et[:, dlo:dlo + 128],
                                        1.0, tri_f,
                                        op0=AX.subtract, op1=AX.mult)
                                    if dlo + 128 < THW:
                                        nc.vector.tensor_scalar(
                                            esb[:, i, t0 + dlo + 128:t0 + THW],
                                            et[:, dlo + 128:], 1.0, None,
                                            op0=AX.subtract)
                                else:
                                    nc.vector.tensor_scalar(
                                        esb[:, i, t0:t0 + THW],
                                        et[:], 1.0, None, op0=AX.subtract)

                        # running prefix column-sums of v_ext
                        cs = pscs.tile([65, TB], FP32, tag="cs")
                        ccol = C_C + (TB + 2) * (h % 2)
                        c_sb = smf[0:65, ccol:ccol + TB + 1]
                        for i in range(TB):
                            nc.tensor.matmul(
                                cs[:, i:i + 1], vsb[:, i, h, :], ones_col,
                                start=True, stop=True)
                        nc.any.memset(c_sb[:, 0:1], 0.0)
                        for i in range(TB):
                            nc.vector.tensor_add(
                                c_sb[:, i + 1:i + 2], c_sb[:, i:i + 1],
                                cs[:, i:i + 1])

                        # o_unnorm^T[(e|Z), t] accumulation (row 64 = Z)
                        ops = pso.tile([65, T], FP32, tag="o_ps")
                        for j in range(TB):
                            jc = slice(j * 128, (j + 1) * 128)
                            nc.tensor.matmul(
                                ops[:, jc], vsb[:, j, h, :], tri_b,
                                start=True, stop=False)
                            for i in range(j + 1):
                                nc.tensor.matmul(
                                    ops[:, jc], vsb[:, i, h, :], esb[:, i, jc],
                                    start=False, stop=(i == j))
                            # Z row to partition 0 of mu_row
                            nc.vector.tensor_scalar(
                                mu_row[:, jc], ops[64:65, jc],
                                c_sb[64:65, j:j + 1], None, op0=AX.add)
                        for th in range(NTH):
                            tsl = slice(th * THW, (th + 1) * THW)
                            nc.vector.reciprocal(r_row[:, tsl], mu_row[:, tsl])
                            nc.gpsimd.partition_broadcast(
                                bc[0:64, tsl], r_row[:, tsl])
                            for j in range(th * (THW // 128),
                                           (th + 1) * (THW // 128)):
                                jc = slice(j * 128, (j + 1) * 128)
                                nc.vector.scalar_tensor_tensor(
                                    osb[:, h, jc], ops[0:64, jc],
                                    c_sb[0:64, j:j + 1], bc[0:64, jc],
                                    op0=AX.add, op1=AX.mult)
                            nc.sync.dma_start(
                                ag_ins[th][:, h, :], osb[:, h, tsl])

                    # ---- AllGather head outputs + full Wo + residual ----
                    ofull = bigp.tile([128, KS, T], BF16, tag="big")
                    for th in range(NTH):
                        tsl = slice(th * THW, (th + 1) * THW)
                        if SKIP_COLLECTIVES:
                            nc.sync.dma_start(ag_outs[th][0], ag_ins[th][:])
                        else:
                            nc.gpsimd.collective_compute(
                                "AllGather", AX.bypass, replica_groups=GROUPS,
                                ins=[ag_ins[th].opt()],
                                outs=[ag_outs[th].opt()])
                        # global o-feature f = 256 r + 64 h + e;
                        # partition = 64 (h%2) + e, subtile = 2 r + h//2
                        ag_v = ag_outs[th][:].rearrange(
                            "r e (h1 h2) t -> h2 r e h1 t", h1=2, h2=2)
                        for h2 in range(2):
                            for r_ in range(TP):
                                nc.sync.dma_start(
                                    ofull[64 * h2:64 * (h2 + 1),
                                          2 * r_:2 * r_ + 2, tsl],
                                    ag_v[h2, r_])
                    for th in range(NTH):
                        tc_ = slice(th * THW, (th + 1) * THW)
                        for m in range(KS):
                            ps = ps512.tile([128, THW], FP32, tag="ps512")
                            for k in range(KS):
                                nc.tensor.matmul(
                                    ps[:], wo[:, k, m * 128:(m + 1) * 128],
                                    ofull[:, k, tc_],
                                    start=(k == 0), stop=(k == KS - 1))
                            nc.vector.scalar_tensor_tensor(
                                x_sb[:, m, tc_], ps[:],
                                bo2_ap(m, l, 0), x_sb[:, m, tc_],
                                op0=AX.add, op1=AX.add)

                    # ---- FFN ----
                    hsb = bigp.tile([128, KS, T], BF16, tag="big")
                    for th in range(NTH):
                        tc_ = slice(th * THW, (th + 1) * THW)
                        for m in range(KS):
                            ps = ps512.tile([128, THW], FP32, tag="ps512")
                            for k in range(KS):
                                nc.tensor.matmul(
                                    ps[:],
                                    w1[:, k, m * 128:(m + 1) * 128],
                                    xr(k, tc_),
                                    start=(k == 0), stop=(k == KS - 1))
                            nc.scalar.activation(
                                hsb[:, m, tc_], ps[:],
                                mybir.ActivationFunctionType.Relu,
                                bias=b1_ap(m, l))
                    ffn_ev = bigp.tile([128, KS, T], BF16, tag="big")
                    ar_ret = bigp.tile([128, KS, T], BF16, tag="big")
                    for th in range(NTH):
                        tc_ = slice(th * THW, (th + 1) * THW)
                        arf_in = dp.tile([128, KS, THW], BF16, tag=f"arf_in{th}")
                        arf_out = dp.tile([128, KS, THW], BF16, tag=f"arf_out{th}")
                        for m in range(KS):
                            ps = ps512.tile([128, THW], FP32, tag="ps512")
                            for k in range(KS):
                                nc.tensor.matmul(
                                    ps[:], w2[:, k, m * 128:(m + 1) * 128],
                                    hsb[:, k, tc_],
                                    start=(k == 0), stop=(k == KS - 1))
                            nc.vector.tensor_copy(ffn_ev[:, m, tc_], ps[:])
                            nc.sync.dma_start(
                                arf_in[:, m, :], ffn_ev[:, m, tc_])
                        if SKIP_COLLECTIVES:
                            nc.sync.dma_start(arf_out[:], arf_in[:])
                        else:
                            nc.gpsimd.collective_compute(
                                "AllReduce", AX.add, replica_groups=GROUPS,
                                ins=[arf_in.opt()], outs=[arf_out.opt()])
                        nc.sync.dma_start(ar_ret[:, :, tc_], arf_out[:])
                        for m in range(KS):
                            nc.vector.scalar_tensor_tensor(
                                x_sb[:, m, tc_], ar_ret[:, m, tc_],
                                bo2_ap(m, l, 1), x_sb[:, m, tc_],
                                op0=AX.add, op1=AX.add)

            # ---- final LayerNorm (feature-major: stats over partitions) ----
            xbl = bigp.tile([128, KS, T], BF16, tag="big")
            for k in range(KS):
                nc.vector.tensor_copy(xbl[:, k, :], x_sb[:, k, :])
            for th in range(NTH):
                tc_ = slice(th * THW, (th + 1) * THW)
                mp = psmu.tile([1, THW], FP32, tag="mu_ps")
                for k in range(KS):
                    nc.tensor.matmul(
                        mp[:], ones_col, xbl[:, k, tc_],
                        start=(k == 0), stop=(k == KS - 1))
                nc.vector.tensor_scalar(
                    mu_row[:, tc_], mp[:], 1.0 / D, None, op0=AX.mult)
            nc.gpsimd.partition_broadcast(bc[:], mu_row[:])
            ysq = bigp.tile([128, KS, T], BF16, tag="big")
            for k in range(KS):
                nc.vector.tensor_sub(x_sb[:, k, :], x_sb[:, k, :], bc[:])
                nc.vector.tensor_mul(ysq[:, k, :], x_sb[:, k, :], x_sb[:, k, :])
            for th in range(NTH):
                tc_ = slice(th * THW, (th + 1) * THW)
                vp = psmu.tile([1, THW], FP32, tag="mu_ps")
                for k in range(KS):
                    nc.tensor.matmul(
                        vp[:], ones_col, ysq[:, k, tc_],
                        start=(k == 0), stop=(k == KS - 1))
                # s = sqrt(var/D + eps)
                nc.scalar.activation(
                    r_row[:, tc_], vp[:],
                    mybir.ActivationFunctionType.Sqrt,
                    bias=smf[0:1, C_EPS:C_EPS + 1], scale=1.0 / D)
            nc.vector.reciprocal(mu_row[:], r_row[:])   # 1/s
            nc.gpsimd.partition_broadcast(bc[:], mu_row[:])
            xln = bigp.tile([128, KS, T], BF16, tag="big")
            for k in range(KS):
                nc.vector.tensor_mul(x_sb[:, k, :], x_sb[:, k, :], bc[:])
                nc.vector.tensor_scalar(
                    xln[:, k, :], x_sb[:, k, :],
                    smf[:, k:k + 1], smf[:, 8 + k:9 + k],
                    op0=AX.mult, op1=AX.add)

            # ---- logits: x_ln @ Wf + bf  (vocab-sharded) ----
            with tc.tile_pool(name="wf_pool", bufs=3) as wfp, \
                 tc.tile_pool(name="lo_pool", bufs=3) as lop:
                for ch in range(1 if SKIP_LOGITS else VT // VCH):
                    wfc = wfp.tile([128, KS, VCH * 128], BF16, tag="wfc")
                    nc.sync.dma_start(
                        wfc[:],
                        d_wf[:, :, ch * VCH * 128:(ch + 1) * VCH * 128])
                    for vt in range(VCH):
                        m = ch * VCH + vt
                        lo = lop.tile([128, T], FP32, tag="lo")
                        for th in range(NTH):
                            tc_ = slice(th * THW, (th + 1) * THW)
                            ps = ps512.tile([128, THW], FP32, tag="ps512")
                            for k in range(KS):
                                nc.tensor.matmul(
                                    ps[:],
                                    wfc[:, k, vt * 128:(vt + 1) * 128],
                                    xln[:, k, tc_],
                                    start=(k == 0), stop=(k == KS - 1))
                            nc.vector.tensor_scalar(
                                lo[:, tc_], ps[:],
                                smf[:, 16 + m:17 + m], None, op0=AX.add)
                        nc.sync.dma_start(d_out[:, m, :], lo[:])

    nc.compile()
    return nc


def _prep_inputs(idx, tok_emb, pos_emb, Wq, Wk, Wv, Wo, bo, W1, b1, W2, b2,
                 ln_g, ln_b, Wf, bf, T, L):
    """Build the 8 per-core input maps (numpy, host-side sharding)."""
    bf16 = ml_dtypes.bfloat16

    def fsplit(a):
        # [D, ...] -> [128, D//128, ...]: feature d -> (d % 128, d // 128)
        return np.ascontiguousarray(
            a.reshape(a.shape[0] // 128, 128, *a.shape[1:]).swapaxes(0, 1))

    tri = np.triu(np.ones((128, 128), np.float32))  # tri[s,t] = 1 if s<=t

    Wf_pad = np.zeros((D, VPAD), np.float32)
    Wf_pad[:, :V] = Wf
    bf_pad = np.zeros((VPAD,), np.float32)
    bf_pad[:V] = bf

    in_maps = []
    for c in range(8):
        g, r = c // TP, c % TP
        x0 = tok_emb[idx[g, :T]] + pos_emb[:T]          # [T, D]
        xT = np.ascontiguousarray(x0.T)                  # [D, T]

        hsel = [4 * r + h_ for h_ in range(HLOC)]
        # wqk [128, KS, L, 2, 2, 128]
        wqk = np.empty((L, 2, 2, D, 128), np.float32)
        for pair in range(2):
            for qk in range(2):
                Wsrc = Wq if qk == 0 else Wk
                wqk[:, pair, qk, :, 0:64] = Wsrc[:L, hsel[2 * pair]]
                wqk[:, pair, qk, :, 64:128] = Wsrc[:L, hsel[2 * pair + 1]]
        wqk = fsplit(wqk.transpose(3, 0, 1, 2, 4))
        # wv [128, KS, L, 256]
        wv = np.concatenate([Wv[:L, h_] for h_ in hsel], axis=-1)  # [L, D, 256]
        wv = fsplit(wv.transpose(1, 0, 2))
        # wo [128, KS, L, 1024] (full Wo)
        wo = fsplit(Wo[:L].transpose(1, 0, 2))
        # w1 column slice, w2 row slice
        w1 = fsplit(W1[:L, :, FLOC * r:FLOC * (r + 1)].transpose(1, 0, 2))
        w2 = fsplit(W2[:L, FLOC * r:FLOC * (r + 1), :].transpose(1, 0, 2))
        # b1s packed [128, KS*L] with column m*L + l
        b1s = fsplit(b1[:L, FLOC * r:FLOC * (r + 1)].T)            # [128, KS, L]
        b1s = b1s.reshape(128, KS * L)
        # bo2 packed [128, KS*2L] with column m*2L + l*2 + {0,1}
        bo2 = fsplit(np.stack([bo[:L].T, b2[:L].T], axis=-1))      # [128, KS, L, 2]
        bo2 = bo2.reshape(128, KS * L * 2)
        wf = fsplit(Wf_pad[:, VLOC * r:VLOC * (r + 1)])
        bfb = bf_pad[VLOC * r:VLOC * (r + 1)].reshape(VLOC // 128, 128).T
        in_maps.append({
            "x0": fsplit(xT).astype(np.float32),
            "wqk": np.ascontiguousarray(wqk, np.float32),
            "wv": np.ascontiguousarray(wv, np.float32),
            "wo": wo.astype(bf16),
            "w1": np.ascontiguousarray(w1, np.float32),
            "w2": w2.astype(bf16),
            "b1s": np.ascontiguousarray(b1s, np.float32),
            "bo2": np.ascontiguousarray(bo2, np.float32),
            "wf": wf.astype(bf16),
            "bfb": np.ascontiguousarray(bfb, np.float32),
            "lng": np.ascontiguousarray(fsplit(ln_g), np.float32),
            "lnb": np.ascontiguousarray(fsplit(ln_b), np.float32),
            "tri": tri,
        })
    return in_maps


def _assemble(results, T):
    """Per-core lg [128, VT, T] f32 -> logits [B, T, V]."""
    logits = np.empty((B, T, V), np.float32)
    for c in range(8):
        g, r = c // TP, c % TP
        lg = results[c]["lg"].reshape(128, VLOC // 128, T)
        block = lg.transpose(2, 1, 0).reshape(T, VLOC)   # v_loc = 128 m + p
        lo = VLOC * r
        hi = min(VLOC * (r + 1), V)
        if lo < V:
            logits[g, :, lo:hi] = block[:, :hi - lo]
    return logits


_CACHE = {}


def kernel(idx, targets, tok_emb, pos_emb, Wq, Wk, Wv, Wo, bo,
           W1, b1, W2, b2, ln_g, ln_b, Wf, bf):
    T, L = 1024, 6
    f = lambda a: np.asarray(a, np.float32)
    idx = np.asarray(idx)
    targets = np.asarray(targets)

    if "nc" not in _CACHE:
        _CACHE["nc"] = build_nc(T, L)
    nc = _CACHE["nc"]

    in_maps = _prep_inputs(idx, f(tok_emb), f(pos_emb), f(Wq), f(Wk), f(Wv),
                           f(Wo), f(bo), f(W1), f(b1), f(W2), f(b2),
                           f(ln_g), f(ln_b), f(Wf), f(bf), T, L)
    res = run_bass_kernel_spmd(nc, in_maps, core_ids=list(range(8)))
    logits = _assemble(res.results, T)

    # loss on host from the device logits (cheap scalar reduction)
    lg64 = logits.astype(np.float64)
    m = lg64.max(axis=-1, keepdims=True)
    lse = np.log(np.exp(lg64 - m).sum(axis=-1, keepdims=True)) + m
    tgt = np.take_along_axis(lg64, targets.astype(np.int64)[..., None], axis=-1)
    loss = np.float32((lse - tgt).mean())
    return logits, loss


# revision 27
# speedup vs baseline: 1.1516x; 1.1516x over previous
"""Trainium2 Bass kernel for nn_BigramLanguageModel (6-layer dense transformer).

Sharding: DP-2 over batch (one die of 4 cores per batch element) x TP-4
Megatron over heads / FFN hidden / vocab within each group.

Layout convention: activations are feature-major ("xT" = [D, T]) and stored
in SBUF as [128, D//128, T] (feature d -> partition d%128, subtile d//128).
All matmuls run in bf16 with fp32 PSUM accumulation. The residual stream
stays fp32 in SBUF.

Attention per (batch, head), with scores transposed (S^T = [s, t]) so the
causal mask is block-triangular and only diagonal 128x128 blocks need
masking. Softmax uses the expm1 decomposition to survive bf16:
  exp(s) = 1 + E' with E' = exp(s) - 1; the scores here are ~1e-3 so E'
  carries the signal at full relative precision in bf16, while the "1"
  part is summed exactly via a triangular-ones matmul plus a running
  column-sum prefix of V_ext = [ones | V] (the ones column puts the
  softmax normalizer on PSUM partition 0). Normalization is delayed past
  the A@V matmul and applied per token via reciprocal + partition
  broadcast, fused into the PSUM eviction.

Per layer cross-core comms: AllGather of the per-rank attention head
outputs (each core then applies the full Wo), and an AllReduce of the
row-split FFN W2 partial outputs. Both in bf16.
"""

import numpy as np
import ml_dtypes

import concourse.bass as bass
import concourse.mybir as mybir
import concourse.tile as tile
from concourse import bacc
from concourse.bass_utils import run_bass_kernel_spmd

FP32 = mybir.dt.float32
BF16 = mybir.dt.bfloat16
AX = mybir.AluOpType
FP32R = mybir.dt.float32r

# Model constants
B, D, H, HD, V = 2, 1024, 16, 64, 32000
KS = D // 128          # 8 feature subtiles
TP = 4                 # tensor-parallel group size
HLOC = H // TP         # 4 heads per core
FLOC = 4 * D // TP     # 1024 ffn hidden per core
VPAD = 32768
VLOC = VPAD // TP      # 8192 vocab columns per core
EPS = 1e-5
SCALE = float(D) ** -0.5

GROUPS = [[0, 1, 2, 3], [4, 5, 6, 7]]

# debug/timing switches (ab_test.py)
SKIP_COLLECTIVES = False
SKIP_LOGITS = False


def build_nc(T=1024, L=6):
    """Build the SPMD program (identical on all 8 cores)."""
    TB = T // 128                      # t/s 128-blocks
    THW = min(512, T)                  # matmul N chunk
    NTH = T // THW                     # n chunks over t
    VT = VLOC // 128                   # 64 vocab 128-tiles
    VCH = 4                            # vocab tiles per wf DMA chunk
    assert T % 128 == 0 and THW % 128 == 0

    nc = bacc.Bacc(None, target_bir_lowering=False, debug=False, num_devices=8)

    # ---- DRAM parameters (per-core shards prepared on host) ----
    d_x0 = nc.declare_dram_parameter("x0", [128, KS, T], FP32R, isOutput=False)
    d_wqk = nc.declare_dram_parameter("wqk", [128, KS, L, 2, 2, 128], FP32R, isOutput=False)
    d_wv = nc.declare_dram_parameter("wv", [128, KS, L, 256], FP32R, isOutput=False)
    d_wo = nc.declare_dram_parameter("wo", [128, KS, L, 1024], BF16, isOutput=False)
    d_w1 = nc.declare_dram_parameter("w1", [128, KS, L, 1024], FP32R, isOutput=False)
    d_w2 = nc.declare_dram_parameter("w2", [128, KS, L, 1024], BF16, isOutput=False)
    d_b1 = nc.declare_dram_parameter("b1s", [128, KS * L], FP32, isOutput=False)
    d_bo2 = nc.declare_dram_parameter("bo2", [128, KS * L * 2], FP32, isOutput=False)
    d_wf = nc.declare_dram_parameter("wf", [128, KS, VLOC], BF16, isOutput=False)
    d_bfb = nc.declare_dram_parameter("bfb", [128, VT], FP32, isOutput=False)
    d_lng = nc.declare_dram_parameter("lng", [128, KS], FP32, isOutput=False)
    d_lnb = nc.declare_dram_parameter("lnb", [128, KS], FP32, isOutput=False)
    d_tri = nc.declare_dram_parameter("tri", [128, 128], FP32, isOutput=False)
    d_out = nc.declare_dram_parameter("lg", [128, VT, T], FP32, isOutput=True)

    with tile.TileContext(nc) as tc:
        with (
            tc.tile_pool(name="persist", bufs=1) as pp,
            tc.tile_pool(name="big", bufs=2) as bigp,
            tc.tile_pool(name="ps512", bufs=4, space="PSUM") as ps512,
            tc.tile_pool(name="ps_o", bufs=2, space="PSUM") as pso,
            tc.tile_pool(name="ps_cs", bufs=1, space="PSUM") as pscs,
            tc.tile_pool(name="ps_mu", bufs=1, space="PSUM") as psmu,
            tc.tile_pool(name="dram", bufs=1, space="DRAM") as dp,
        ):
            # ---- persistent SBUF state ----
            x_sb = pp.tile([128, KS, T], FP32R, name="x_sb")
            qk_sb = pp.tile([128, 4, T], BF16, name="qk_sb")  # q pairs 0-1, k pairs 2-3
            vsb = pp.tile([128, TB, HLOC, 65], BF16, name="vsb")  # col 64 = ones
            osb = pp.tile([64, HLOC, T], BF16, name="osb")
            r_row = pp.tile([1, T], FP32, name="r_row")
            zoff = pp.tile([1, T], FP32, name="zoff")
            mu_row = pp.tile([1, T], FP32, name="mu_row")
            bc = pp.tile([128, T], FP32, name="bc")
            # packed f32 scalar/constant columns
            C_B1 = 208
            C_BO2 = C_B1 + KS * L
            C_EPS = C_BO2 + KS * 2 * L
            C_M1 = C_EPS + 1
            C_Z = C_M1 + 1
            C_C = C_Z + 1
            smf = pp.tile([128, C_C + 2 * (TB + 2)], FP32, name="smf")
            # packed bf16: 0:128 tri_b | 128:129 ones_col
            smb = pp.tile([128, 132], BF16, name="smb")

            tri_f = smf[:, 80:208]
            tri_b = smb[:, 0:128]
            ones_col = smb[:, 128:129]

            nc.sync.dma_start(x_sb[:], d_x0[:])
            nc.sync.dma_start(smf[:, 80:208], d_tri[:])
            nc.sync.dma_start(smf[:, C_B1:C_B1 + KS * L], d_b1[:])
            nc.sync.dma_start(smf[:, C_BO2:C_BO2 + KS * 2 * L], d_bo2[:])
            nc.sync.dma_start(smf[:, 0:8], d_lng[:])
            nc.sync.dma_start(smf[:, 8:16], d_lnb[:])
            nc.sync.dma_start(smf[:, 16:80], d_bfb[:])
            nc.vector.tensor_copy(tri_b, tri_f)
            nc.any.memset(ones_col, 1.0)
            nc.any.memset(smf[0:1, C_EPS:C_EPS + 1], EPS)
            nc.any.memset(smf[:, C_M1:C_M1 + 1], -1.0)
            nc.any.memset(smf[:, C_Z:C_Z + 1], 0.0)
            nc.any.memset(vsb[:, :, :, 64:65], 1.0)
            for j in range(TB):
                nc.any.memset(zoff[:, j * 128:(j + 1) * 128], float(128 * j))

            def xr(k, tsl):
                return x_sb[:, k, tsl]

            def b1_ap(m, l):
                c0 = C_B1 + m * L + l
                return smf[:, c0:c0 + 1]

            def bo2_ap(m, l, which):
                c0 = C_BO2 + m * 2 * L + l * 2 + which
                return smf[:, c0:c0 + 1]

            with (
                tc.tile_pool(name="wq_pool", bufs=1) as wqp,
                tc.tile_pool(name="wo_pool", bufs=1) as wop,
                tc.tile_pool(name="w1_pool", bufs=1) as w1p,
                tc.tile_pool(name="w2_pool", bufs=1) as w2p,
                tc.tile_pool(name="etmp_pool", bufs=2) as etp,
            ):
                for l in range(L):
                    # ---- layer weight loads (slot reuse staggers these) ----
                    wqk = wqp.tile([128, KS, 2, 2, 128], FP32R, tag="wqk")
                    wv = wqp.tile([128, KS, 256], FP32R, tag="wv")
                    wo = wop.tile([128, KS, 1024], BF16, tag="wo")
                    w1 = w1p.tile([128, KS, 1024], FP32R, tag="w1")
                    w2 = w2p.tile([128, KS, 1024], BF16, tag="w2")
                    nc.sync.dma_start(wqk[:], d_wqk[:, :, l])
                    nc.sync.dma_start(wv[:], d_wv[:, :, l])
                    nc.sync.dma_start(wo[:], d_wo[:, :, l])
                    nc.sync.dma_start(w1[:], d_w1[:, :, l])
                    nc.sync.dma_start(w2[:], d_w2[:, :, l])

                    # ---- QKV projections (t-half major) ----
                    for th in range(NTH):
                        tsl = slice(th * THW, (th + 1) * THW)
                        for pair in range(2):
                            for qk in range(2):
                                ps = ps512.tile([128, THW], FP32, tag="ps512")
                                for k in range(KS):
                                    nc.tensor.matmul(
                                        ps[:],
                                        wqk[:, k, pair, qk, :],
                                        xr(k, tsl),
                                        start=(k == 0), stop=(k == KS - 1))
                                nc.vector.tensor_copy(
                                    qk_sb[:, 2 * qk + pair, tsl], ps[:])
                        for tb in range(th * (THW // 128),
                                        (th + 1) * (THW // 128)):
                            ps = ps512.tile([128, THW], FP32, tag="ps512")
                            for k in range(KS):
                                nc.tensor.matmul(
                                    ps[:, 0:256],
                                    xr(k, slice(tb * 128, (tb + 1) * 128)),
                                    wv[:, k, :],
                                    start=(k == 0), stop=(k == KS - 1))
                            nc.vector.tensor_copy(
                                vsb[:, tb, :, 0:64],
                                ps[:, 0:256].rearrange("p (h e) -> p h e", e=64))

                    # ---- attention per local head ----
                    ag_ins = [dp.tile([64, HLOC, THW], BF16, tag=f"ag_in{th}",
                                      name=f"ag_in{th}") for th in range(NTH)]
                    ag_outs = [dp.tile([TP, 64, HLOC, THW], BF16,
                                       tag=f"ag_out{th}", name=f"ag_out{th}")
                               for th in range(NTH)]
                    for h in range(HLOC):
                        hp = 64 * (h % 2)
                        hs = h // 2
                        esb = bigp.tile([128, TB, T], BF16, tag="big")
                        # scores S^T[s,t] -> E' = exp(s*SCALE)-1 (masked)
                        for i in range(TB):
                            for th in range(NTH):
                                t0 = th * THW
                                if t0 + THW <= 128 * i:
                                    continue  # fully below diagonal: unused
                                sc = ps512.tile([128, THW], FP32, tag="ps512")
                                nc.tensor.matmul(
                                    sc[:],
                                    qk_sb[hp:hp + 64, 2 + hs,
                                          i * 128:(i + 1) * 128],
                                    qk_sb[hp:hp + 64, hs, t0:t0 + THW],
                                    start=True, stop=True)
                                et = etp.tile([128, THW], FP32, tag="etmp")
                                nc.scalar.activation(
                                    et[:], sc[:],
                                    mybir.ActivationFunctionType.Exp,
                                    scale=SCALE)
                                # diagonal 128-block inside this (i, th)?
                                dlo = i * 128 - t0
                                if 0 <= dlo < THW:
                                    nc.vector.scalar_tensor_tensor(
                                        esb[:, i, i * 128:(i + 1) * 128],
                                        et[:, dlo:dlo + 128],
                                        1.0, tri_f,
                                        op0=AX.subtract, op1=AX.mult)
                                    if dlo + 128 < THW:
                                        nc.vector.tensor_scalar(
                                            esb[:, i, t0 + dlo + 128:t0 + THW],
                                            et[:, dlo + 128:], 1.0, None,
                                            op0=AX.subtract)
                                else:
                                    nc.vector.tensor_scalar(
                                        esb[:, i, t0:t0 + THW],
                                        et[:], 1.0, None, op0=AX.subtract)

                        # running prefix column-sums of v_ext
                        cs = pscs.tile([65, TB], FP32, tag="cs")
                        ccol = C_C + (TB + 2) * (h % 2)
                        c_sb = smf[0:65, ccol:ccol + TB + 1]
                        for i in range(TB):
                            nc.tensor.matmul(
                                cs[:, i:i + 1], vsb[:, i, h, :], ones_col,
                                start=True, stop=True)
                        nc.any.memset(c_sb[:, 0:1], 0.0)
                        for i in range(TB):
                            nc.vector.tensor_add(
                                c_sb[:, i + 1:i + 2], c_sb[:, i:i + 1],
                                cs[:, i:i + 1])

                        # o_unnorm^T[(e|Z), t] accumulation (row 64 = Z)
                        for th in range(NTH):
                            tsl = slice(th * THW, (th + 1) * THW)
                            ops = pso.tile([65, THW], FP32, tag="o_ps")
                            for j in range(th * (THW // 128),
                                           (th + 1) * (THW // 128)):
                                jl = slice((j * 128) % THW,
                                           (j * 128) % THW + 128)
                                jc = slice(j * 128, (j + 1) * 128)
                                nc.tensor.matmul(
                                    ops[:, jl], vsb[:, j, h, :], tri_b,
                                    start=True, stop=False)
                                for i in range(j + 1):
                                    nc.tensor.matmul(
                                        ops[:, jl], vsb[:, i, h, :],
                                        esb[:, i, jc],
                                        start=False, stop=(i == j))
                            nc.vector.tensor_add(
                                mu_row[:, tsl], ops[64:65, :], zoff[:, tsl])
                            nc.vector.reciprocal(r_row[:, tsl], mu_row[:, tsl])
                            nc.gpsimd.partition_broadcast(
                                bc[0:64, tsl], r_row[:, tsl])
                            for j in range(th * (THW // 128),
                                           (th + 1) * (THW // 128)):
                                jl = slice((j * 128) % THW,
                                           (j * 128) % THW + 128)
                                jc = slice(j * 128, (j + 1) * 128)
                                nc.vector.scalar_tensor_tensor(
                                    osb[:, h, jc], ops[0:64, jl],
                                    c_sb[0:64, j:j + 1], bc[0:64, jc],
                                    op0=AX.add, op1=AX.mult)
                            nc.sync.dma_start(
                                ag_ins[th][:, h, :], osb[:, h, tsl])

                    # ---- AllGather head outputs + full Wo + residual ----
                    ofull = bigp.tile([128, KS, T], BF16, tag="big")
                    for th in range(NTH):
                        tsl = slice(th * THW, (th + 1) * THW)
                        if SKIP_COLLECTIVES:
                            nc.sync.dma_start(ag_outs[th][0], ag_ins[th][:])
                        else:
                            nc.gpsimd.collective_compute(
                                "AllGather", AX.bypass, replica_groups=GROUPS,
                                ins=[ag_ins[th].opt()],
                                outs=[ag_outs[th].opt()])
                        # global o-feature f = 256 r + 64 h + e;
                        # partition = 64 (h%2) + e, subtile = 2 r + h//2
                        ag_v = ag_outs[th][:].rearrange(
                            "r e (h1 h2) t -> h2 r e h1 t", h1=2, h2=2)
                        for h2 in range(2):
                            for r_ in range(TP):
                                nc.sync.dma_start(
                                    ofull[64 * h2:64 * (h2 + 1),
                                          2 * r_:2 * r_ + 2, tsl],
                                    ag_v[h2, r_])
                    for th in range(NTH):
                        tc_ = slice(th * THW, (th + 1) * THW)
                        for m in range(KS):
                            ps = ps512.tile([128, THW], FP32, tag="ps512")
                            for k in range(KS):
                                nc.tensor.matmul(
                                    ps[:], wo[:, k, m * 128:(m + 1) * 128],
                                    ofull[:, k, tc_],
                                    start=(k == 0), stop=(k == KS - 1))
                            nc.vector.scalar_tensor_tensor(
                                x_sb[:, m, tc_], ps[:],
                                bo2_ap(m, l, 0), x_sb[:, m, tc_],
                                op0=AX.add, op1=AX.add)

                    # ---- FFN ----
                    hsb = bigp.tile([128, KS, T], BF16, tag="big")
                    for th in range(NTH):
                        tc_ = slice(th * THW, (th + 1) * THW)
                        for m in range(KS):
                            ps = ps512.tile([128, THW], FP32, tag="ps512")
                            for k in range(KS):
                                nc.tensor.matmul(
                                    ps[:],
                                    w1[:, k, m * 128:(m + 1) * 128],
                                    xr(k, tc_),
                                    start=(k == 0), stop=(k == KS - 1))
                            nc.scalar.activation(
                                hsb[:, m, tc_], ps[:],
                                mybir.ActivationFunctionType.Relu,
                                bias=b1_ap(m, l))
                    ffn_ev = bigp.tile([128, KS, T], BF16, tag="big")
                    ar_ret = bigp.tile([128, KS, T], BF16, tag="big")
                    if l == L - 1:
                        xbl = bigp.tile([128, KS, T], BF16, tag="big",
                                        name="xbl")
                    for th in range(NTH):
                        tc_ = slice(th * THW, (th + 1) * THW)
                        arf_in = dp.tile([128, KS, THW], BF16, tag=f"arf_in{th}")
                        arf_out = dp.tile([128, KS, THW], BF16, tag=f"arf_out{th}")
                        for m in range(KS):
                            ps = ps512.tile([128, THW], FP32, tag="ps512")
                            for k in range(KS):
                                nc.tensor.matmul(
                                    ps[:], w2[:, k, m * 128:(m + 1) * 128],
                                    hsb[:, k, tc_],
                                    start=(k == 0), stop=(k == KS - 1))
                            nc.vector.tensor_copy(ffn_ev[:, m, tc_], ps[:])
                            nc.sync.dma_start(
                                arf_in[:, m, :], ffn_ev[:, m, tc_])
                        if SKIP_COLLECTIVES:
                            nc.sync.dma_start(arf_out[:], arf_in[:])
                        else:
                            nc.gpsimd.collective_compute(
                                "AllReduce", AX.add, replica_groups=GROUPS,
                                ins=[arf_in.opt()], outs=[arf_out.opt()])
                        nc.sync.dma_start(ar_ret[:, :, tc_], arf_out[:])
                        for m in range(KS):
                            nc.vector.scalar_tensor_tensor(
                                x_sb[:, m, tc_], ar_ret[:, m, tc_],
                                bo2_ap(m, l, 1), x_sb[:, m, tc_],
                                op0=AX.add, op1=AX.add)
                            if l == L - 1:
                                nc.vector.tensor_copy(
                                    xbl[:, m, tc_], x_sb[:, m, tc_])

            # ---- final LayerNorm (feature-major: stats over partitions) ----
            for th in range(NTH):
                tc_ = slice(th * THW, (th + 1) * THW)
                mp = psmu.tile([1, THW], FP32, tag="mu_ps")
                for k in range(KS):
                    nc.tensor.matmul(
                        mp[:], ones_col, xbl[:, k, tc_],
                        start=(k == 0), stop=(k == KS - 1))
                nc.vector.tensor_scalar(
                    mu_row[:, tc_], mp[:], 1.0 / D, None, op0=AX.mult)
            nc.gpsimd.partition_broadcast(bc[:], mu_row[:])
            ysq = bigp.tile([128, KS, T], BF16, tag="big")
            for k in range(KS):
                nc.vector.tensor_sub(x_sb[:, k, :], x_sb[:, k, :], bc[:])
                nc.vector.tensor_mul(ysq[:, k, :], x_sb[:, k, :], x_sb[:, k, :])
            for th in range(NTH):
                tc_ = slice(th * THW, (th + 1) * THW)
                vp = psmu.tile([1, THW], FP32, tag="mu_ps")
                for k in range(KS):
                    nc.tensor.matmul(
                        vp[:], ones_col, ysq[:, k, tc_],
                        start=(k == 0), stop=(k == KS - 1))
                # s = sqrt(var/D + eps)
                nc.scalar.activation(
                    r_row[:, tc_], vp[:],
                    mybir.ActivationFunctionType.Sqrt,
                    bias=smf[0:1, C_EPS:C_EPS + 1], scale=1.0 / D)
            nc.vector.reciprocal(mu_row[:], r_row[:])   # 1/s
            nc.gpsimd.partition_broadcast(bc[:], mu_row[:])
            xln = bigp.tile([128, KS, T], BF16, tag="big")
            for k in range(KS):
                nc.vector.tensor_mul(x_sb[:, k, :], x_sb[:, k, :], bc[:])
                nc.vector.tensor_scalar(
                    xln[:, k, :], x_sb[:, k, :],
                    smf[:, k:k + 1], smf[:, 8 + k:9 + k],
                    op0=AX.mult, op1=AX.add)

            # ---- logits: x_ln @ Wf + bf  (vocab-sharded) ----
            with tc.tile_pool(name="wf_pool", bufs=4) as wfp, \
                 tc.tile_pool(name="lo_pool", bufs=3) as lop:
                for ch in range(1 if SKIP_LOGITS else VT // VCH):
                    wfc = wfp.tile([128, KS, VCH * 128], BF16, tag="wfc")
                    nc.sync.dma_start(
                        wfc[:],
                        d_wf[:, :, ch * VCH * 128:(ch + 1) * VCH * 128])
                    for vt in range(VCH):
                        m = ch * VCH + vt
                        lo = lop.tile([128, T], FP32, tag="lo")
                        for th in range(NTH):
                            tc_ = slice(th * THW, (th + 1) * THW)
                            ps = ps512.tile([128, THW], FP32, tag="ps512")
                            for k in range(KS):
                                nc.tensor.matmul(
                                    ps[:],
                                    wfc[:, k, vt * 128:(vt + 1) * 128],
                                    xln[:, k, tc_],
                                    start=(k == 0), stop=(k == KS - 1))
                            nc.vector.tensor_scalar(
                                lo[:, tc_], ps[:],
                                smf[:, 16 + m:17 + m], None, op0=AX.add)
                        nc.sync.dma_start(d_out[:, m, :], lo[:])

    nc.compile()
    return nc


def _prep_inputs(idx, tok_emb, pos_emb, Wq, Wk, Wv, Wo, bo, W1, b1, W2, b2,
                 ln_g, ln_b, Wf, bf, T, L):
    """Build the 8 per-core input maps (numpy, host-side sharding)."""
    bf16 = ml_dtypes.bfloat16

    def fsplit(a):
        # [D, ...] -> [128, D//128, ...]: feature d -> (d % 128, d // 128)
        return np.ascontiguousarray(
            a.reshape(a.shape[0] // 128, 128, *a.shape[1:]).swapaxes(0, 1))

    tri = np.triu(np.ones((128, 128), np.float32))  # tri[s,t] = 1 if s<=t

    Wf_pad = np.zeros((D, VPAD), np.float32)
    Wf_pad[:, :V] = Wf
    bf_pad = np.zeros((VPAD,), np.float32)
    bf_pad[:V] = bf

    in_maps = []
    for c in range(8):
        g, r = c // TP, c % TP
        x0 = tok_emb[idx[g, :T]] + pos_emb[:T]          # [T, D]
        xT = np.ascontiguousarray(x0.T)                  # [D, T]

        hsel = [4 * r + h_ for h_ in range(HLOC)]
        # wqk [128, KS, L, 2, 2, 128]
        wqk = np.empty((L, 2, 2, D, 128), np.float32)
        for pair in range(2):
            for qk in range(2):
                Wsrc = Wq if qk == 0 else Wk
                wqk[:, pair, qk, :, 0:64] = Wsrc[:L, hsel[2 * pair]]
                wqk[:, pair, qk, :, 64:128] = Wsrc[:L, hsel[2 * pair + 1]]
        wqk = fsplit(wqk.transpose(3, 0, 1, 2, 4))
        # wv [128, KS, L, 256]
        wv = np.concatenate([Wv[:L, h_] for h_ in hsel], axis=-1)  # [L, D, 256]
        wv = fsplit(wv.transpose(1, 0, 2))
        # wo [128, KS, L, 1024] (full Wo)
        wo = fsplit(Wo[:L].transpose(1, 0, 2))
        # w1 column slice, w2 row slice
        w1 = fsplit(W1[:L, :, FLOC * r:FLOC * (r + 1)].transpose(1, 0, 2))
        w2 = fsplit(W2[:L, FLOC * r:FLOC * (r + 1), :].transpose(1, 0, 2))
        # b1s packed [128, KS*L] with column m*L + l
        b1s = fsplit(b1[:L, FLOC * r:FLOC * (r + 1)].T)            # [128, KS, L]
        b1s = b1s.reshape(128, KS * L)
        # bo2 packed [128, KS*2L] with column m*2L + l*2 + {0,1}
        bo2 = fsplit(np.stack([bo[:L].T, b2[:L].T], axis=-1))      # [128, KS, L, 2]
        bo2 = bo2.reshape(128, KS * L * 2)
        wf = fsplit(Wf_pad[:, VLOC * r:VLOC * (r + 1)])
        bfb = bf_pad[VLOC * r:VLOC * (r + 1)].reshape(VLOC // 128, 128).T
        in_maps.append({
            "x0": fsplit(xT).astype(np.float32),
            "wqk": np.ascontiguousarray(wqk, np.float32),
            "wv": np.ascontiguousarray(wv, np.float32),
            "wo": wo.astype(bf16),
            "w1": np.ascontiguousarray(w1, np.float32),
            "w2": w2.astype(bf16),
            "b1s": np.ascontiguousarray(b1s, np.float32),
            "bo2": np.ascontiguousarray(bo2, np.float32),
            "wf": wf.astype(bf16),
            "bfb": np.ascontiguousarray(bfb, np.float32),
            "lng": np.ascontiguousarray(fsplit(ln_g), np.float32),
            "lnb": np.ascontiguousarray(fsplit(ln_b), np.float32),
            "tri": tri,
        })
    return in_maps


def _assemble(results, T):
    """Per-core lg [128, VT, T] f32 -> logits [B, T, V]."""
    logits = np.empty((B, T, V), np.float32)
    for c in range(8):
        g, r = c // TP, c % TP
        lg = results[c]["lg"].reshape(128, VLOC // 128, T)
        block = lg.transpose(2, 1, 0).reshape(T, VLOC)   # v_loc = 128 m + p
        lo = VLOC * r
        hi = min(VLOC * (r + 1), V)
        if lo < V:
            logits[g, :, lo:hi] = block[:, :hi - lo]
    return logits


_CACHE = {}


def kernel(idx, targets, tok_emb, pos_emb, Wq, Wk, Wv, Wo, bo,
           W1, b1, W2, b2, ln_g, ln_b, Wf, bf):
    T, L = 1024, 6
    f = lambda a: np.asarray(a, np.float32)
    idx = np.asarray(idx)
    targets = np.asarray(targets)

    if "nc" not in _CACHE:
        _CACHE["nc"] = build_nc(T, L)
    nc = _CACHE["nc"]

    in_maps = _prep_inputs(idx, f(tok_emb), f(pos_emb), f(Wq), f(Wk), f(Wv),
                           f(Wo), f(bo), f(W1), f(b1), f(W2), f(b2),
                           f(ln_g), f(ln_b), f(Wf), f(bf), T, L)
    res = run_bass_kernel_spmd(nc, in_maps, core_ids=list(range(8)))
    logits = _assemble(res.results, T)

    # loss on host from the device logits (cheap scalar reduction)
    lg64 = logits.astype(np.float64)
    m = lg64.max(axis=-1, keepdims=True)
    lse = np.log(np.exp(lg64 - m).sum(axis=-1, keepdims=True)) + m
    tgt = np.take_along_axis(lg64, targets.astype(np.int64)[..., None], axis=-1)
    loss = np.float32((lse - tgt).mean())
    return logits, loss


# revision 29
# speedup vs baseline: 1.2713x; 1.1039x over previous
"""Trainium2 Bass kernel for nn_BigramLanguageModel (6-layer dense transformer).

Sharding: DP-2 over batch (one die of 4 cores per batch element) x TP-4
Megatron over heads / FFN hidden / vocab within each group.

Layout convention: activations are feature-major ("xT" = [D, T]) and stored
in SBUF as [128, D//128, T] (feature d -> partition d%128, subtile d//128).
All matmuls run in bf16 with fp32 PSUM accumulation. The residual stream
stays fp32 in SBUF.

Attention per (batch, head), with scores transposed (S^T = [s, t]) so the
causal mask is block-triangular and only diagonal 128x128 blocks need
masking. Softmax uses the expm1 decomposition to survive bf16:
  exp(s) = 1 + E' with E' = exp(s) - 1; the scores here are ~1e-3 so E'
  carries the signal at full relative precision in bf16, while the "1"
  part is summed exactly via a triangular-ones matmul plus a running
  column-sum prefix of V_ext = [ones | V] (the ones column puts the
  softmax normalizer on PSUM partition 0). Normalization is delayed past
  the A@V matmul and applied per token via reciprocal + partition
  broadcast, fused into the PSUM eviction.

Per layer cross-core comms: AllGather of the per-rank attention head
outputs (each core then applies the full Wo), and an AllReduce of the
row-split FFN W2 partial outputs. Both in bf16.
"""

import numpy as np
import ml_dtypes

import concourse.bass as bass
import concourse.mybir as mybir
import concourse.tile as tile
from concourse import bacc
from concourse.bass_utils import run_bass_kernel_spmd

FP32 = mybir.dt.float32
BF16 = mybir.dt.bfloat16
AX = mybir.AluOpType
FP32R = mybir.dt.float32r

# Model constants
B, D, H, HD, V = 2, 1024, 16, 64, 32000
KS = D // 128          # 8 feature subtiles
TP = 4                 # tensor-parallel group size
HLOC = H // TP         # 4 heads per core
FLOC = 4 * D // TP     # 1024 ffn hidden per core
VPAD = 32768
VLOC = VPAD // TP      # 8192 vocab columns per core
EPS = 1e-5
SCALE = float(D) ** -0.5

GROUPS = [[0, 1, 2, 3], [4, 5, 6, 7]]

# debug/timing switches (ab_test.py)
SKIP_COLLECTIVES = False
SKIP_LOGITS = False


def build_nc(T=1024, L=6):
    """Build the SPMD program (identical on all 8 cores)."""
    TB = T // 128                      # t/s 128-blocks
    THW = min(512, T)                  # matmul N chunk
    NTH = T // THW                     # n chunks over t
    VT = VLOC // 128                   # 64 vocab 128-tiles
    VCH = 4                            # vocab tiles per wf DMA chunk
    assert T % 128 == 0 and THW % 128 == 0

    nc = bacc.Bacc(None, target_bir_lowering=False, debug=False, num_devices=8)

    # ---- DRAM parameters (per-core shards prepared on host) ----
    d_x0 = nc.declare_dram_parameter("x0", [128, KS, T], FP32R, isOutput=False)
    d_wqk = nc.declare_dram_parameter("wqk", [128, KS, L, 2, 2, 128], FP32R, isOutput=False)
    d_wv = nc.declare_dram_parameter("wv", [128, KS, L, 256], FP32R, isOutput=False)
    d_wo = nc.declare_dram_parameter("wo", [128, KS, L, 1024], BF16, isOutput=False)
    d_w1 = nc.declare_dram_parameter("w1", [128, KS, L, 1024], FP32R, isOutput=False)
    d_w2 = nc.declare_dram_parameter("w2", [128, KS, L, 1024], BF16, isOutput=False)
    d_b1 = nc.declare_dram_parameter("b1s", [128, KS * L], FP32, isOutput=False)
    d_bo2 = nc.declare_dram_parameter("bo2", [128, KS * L * 2], FP32, isOutput=False)
    d_wf = nc.declare_dram_parameter("wf", [128, KS, VLOC], BF16, isOutput=False)
    d_bfb = nc.declare_dram_parameter("bfb", [128, VT], FP32, isOutput=False)
    d_lng = nc.declare_dram_parameter("lng", [128, KS], FP32, isOutput=False)
    d_lnb = nc.declare_dram_parameter("lnb", [128, KS], FP32, isOutput=False)
    d_tri = nc.declare_dram_parameter("tri", [128, 128], FP32, isOutput=False)
    d_out = nc.declare_dram_parameter("lg", [128, VT, T], FP32, isOutput=True)

    with tile.TileContext(nc) as tc:
        with (
            tc.tile_pool(name="persist", bufs=1) as pp,
            tc.tile_pool(name="big", bufs=2) as bigp,
            tc.tile_pool(name="ps512", bufs=4, space="PSUM") as ps512,
            tc.tile_pool(name="ps_o", bufs=2, space="PSUM") as pso,
            tc.tile_pool(name="ps_cs", bufs=1, space="PSUM") as pscs,
            tc.tile_pool(name="ps_mu", bufs=1, space="PSUM") as psmu,
            tc.tile_pool(name="dram", bufs=1, space="DRAM") as dp,
        ):
            # ---- persistent SBUF state ----
            x_sb = pp.tile([128, KS, T], FP32R, name="x_sb")
            qk_sb = pp.tile([128, 4, T], BF16, name="qk_sb")  # q pairs 0-1, k pairs 2-3
            vsb = pp.tile([128, TB, HLOC, 65], BF16, name="vsb")  # col 64 = ones
            osb = pp.tile([64, HLOC, T], BF16, name="osb")
            r_row = pp.tile([1, T], FP32, name="r_row")
            zoff = pp.tile([1, T], FP32, name="zoff")
            mu_row = pp.tile([1, T], FP32, name="mu_row")
            bc = pp.tile([128, T], FP32, name="bc")
            # packed f32 scalar/constant columns
            C_B1 = 208
            C_BO2 = C_B1 + KS * L
            C_EPS = C_BO2 + KS * 2 * L
            C_M1 = C_EPS + 1
            C_Z = C_M1 + 1
            C_C = C_Z + 1
            smf = pp.tile([128, C_C + 2 * (TB + 2)], FP32, name="smf")
            # packed bf16: 0:128 tri_b | 128:129 ones_col
            smb = pp.tile([128, 132], BF16, name="smb")

            tri_f = smf[:, 80:208]
            tri_b = smb[:, 0:128]
            ones_col = smb[:, 128:129]

            nc.sync.dma_start(x_sb[:], d_x0[:])
            nc.sync.dma_start(smf[:, 80:208], d_tri[:])
            nc.sync.dma_start(smf[:, C_B1:C_B1 + KS * L], d_b1[:])
            nc.sync.dma_start(smf[:, C_BO2:C_BO2 + KS * 2 * L], d_bo2[:])
            nc.sync.dma_start(smf[:, 0:8], d_lng[:])
            nc.sync.dma_start(smf[:, 8:16], d_lnb[:])
            nc.sync.dma_start(smf[:, 16:80], d_bfb[:])
            nc.vector.tensor_copy(tri_b, tri_f)
            nc.any.memset(ones_col, 1.0)
            nc.any.memset(smf[0:1, C_EPS:C_EPS + 1], EPS)
            nc.any.memset(smf[:, C_M1:C_M1 + 1], -1.0)
            nc.any.memset(smf[:, C_Z:C_Z + 1], 0.0)
            nc.any.memset(vsb[:, :, :, 64:65], 1.0)
            for j in range(TB):
                nc.any.memset(zoff[:, j * 128:(j + 1) * 128], float(128 * j))

            def xr(k, tsl):
                return x_sb[:, k, tsl]

            def b1_ap(m, l):
                c0 = C_B1 + m * L + l
                return smf[:, c0:c0 + 1]

            def bo2_ap(m, l, which):
                c0 = C_BO2 + m * 2 * L + l * 2 + which
                return smf[:, c0:c0 + 1]

            with (
                tc.tile_pool(name="wq_pool", bufs=1) as wqp,
                tc.tile_pool(name="wo_pool", bufs=1) as wop,
                tc.tile_pool(name="w1_pool", bufs=1) as w1p,
                tc.tile_pool(name="w2_pool", bufs=1) as w2p,
                tc.tile_pool(name="etmp_pool", bufs=2) as etp,
            ):
                for l in range(L):
                    # ---- layer weight loads (slot reuse staggers these) ----
                    wqk = wqp.tile([128, KS, 2, 2, 128], FP32R, tag="wqk")
                    wv = wqp.tile([128, KS, 256], FP32R, tag="wv")
                    wo = wop.tile([128, KS, 1024], BF16, tag="wo")
                    w1 = w1p.tile([128, KS, 1024], FP32R, tag="w1")
                    w2 = w2p.tile([128, KS, 1024], BF16, tag="w2")
                    nc.sync.dma_start(wqk[:], d_wqk[:, :, l])
                    nc.sync.dma_start(wv[:], d_wv[:, :, l])
                    nc.sync.dma_start(wo[:], d_wo[:, :, l])
                    nc.sync.dma_start(w1[:], d_w1[:, :, l])
                    nc.sync.dma_start(w2[:], d_w2[:, :, l])

                    # ---- QKV projections (t-half major) ----
                    for th in range(NTH):
                        tsl = slice(th * THW, (th + 1) * THW)
                        for pair in range(2):
                            for qk in range(2):
                                ps = ps512.tile([128, THW], FP32, tag="ps512")
                                for k in range(KS):
                                    nc.tensor.matmul(
                                        ps[:],
                                        wqk[:, k, pair, qk, :],
                                        xr(k, tsl),
                                        start=(k == 0), stop=(k == KS - 1))
                                nc.vector.tensor_copy(
                                    qk_sb[:, 2 * qk + pair, tsl], ps[:])
                        for tb in range(th * (THW // 128),
                                        (th + 1) * (THW // 128)):
                            ps = ps512.tile([128, THW], FP32, tag="ps512")
                            for k in range(KS):
                                nc.tensor.matmul(
                                    ps[:, 0:256],
                                    xr(k, slice(tb * 128, (tb + 1) * 128)),
                                    wv[:, k, :],
                                    start=(k == 0), stop=(k == KS - 1))
                            nc.vector.tensor_copy(
                                vsb[:, tb, :, 0:64],
                                ps[:, 0:256].rearrange("p (h e) -> p h e", e=64))

                    # ---- attention per local head ----
                    ag_ins = [[dp.tile([64, 2, THW], BF16,
                                       tag=f"ag_in{p_}{th}",
                                       name=f"ag_in{p_}{th}")
                               for th in range(NTH)] for p_ in range(2)]
                    ag_outs = [[dp.tile([TP, 64, 2, THW], BF16,
                                        tag=f"ag_out{p_}{th}",
                                        name=f"ag_out{p_}{th}")
                                for th in range(NTH)] for p_ in range(2)]
                    for h in range(HLOC):
                        hp = 64 * (h % 2)
                        hs = h // 2
                        esb = bigp.tile([128, TB, T], BF16, tag="big")
                        # scores S^T[s,t] -> E' = exp(s*SCALE)-1 (masked)
                        for i in range(TB):
                            for th in range(NTH):
                                t0 = th * THW
                                if t0 + THW <= 128 * i:
                                    continue  # fully below diagonal: unused
                                sc = ps512.tile([128, THW], FP32, tag="ps512")
                                nc.tensor.matmul(
                                    sc[:],
                                    qk_sb[hp:hp + 64, 2 + hs,
                                          i * 128:(i + 1) * 128],
                                    qk_sb[hp:hp + 64, hs, t0:t0 + THW],
                                    start=True, stop=True)
                                et = etp.tile([128, THW], FP32, tag="etmp")
                                nc.scalar.activation(
                                    et[:], sc[:],
                                    mybir.ActivationFunctionType.Exp,
                                    scale=SCALE)
                                # diagonal 128-block inside this (i, th)?
                                dlo = i * 128 - t0
                                if 0 <= dlo < THW:
                                    nc.vector.scalar_tensor_tensor(
                                        esb[:, i, i * 128:(i + 1) * 128],
                                        et[:, dlo:dlo + 128],
                                        1.0, tri_f,
                                        op0=AX.subtract, op1=AX.mult)
                                    if dlo + 128 < THW:
                                        nc.vector.tensor_scalar(
                                            esb[:, i, t0 + dlo + 128:t0 + THW],
                                            et[:, dlo + 128:], 1.0, None,
                                            op0=AX.subtract)
                                else:
                                    nc.vector.tensor_scalar(
                                        esb[:, i, t0:t0 + THW],
                                        et[:], 1.0, None, op0=AX.subtract)

                        # running prefix column-sums of v_ext
                        cs = pscs.tile([65, TB], FP32, tag="cs")
                        ccol = C_C + (TB + 2) * (h % 2)
                        c_sb = smf[0:65, ccol:ccol + TB + 1]
                        for i in range(TB):
                            nc.tensor.matmul(
                                cs[:, i:i + 1], vsb[:, i, h, :], ones_col,
                                start=True, stop=True)
                        nc.any.memset(c_sb[:, 0:1], 0.0)
                        for i in range(TB):
                            nc.vector.tensor_add(
                                c_sb[:, i + 1:i + 2], c_sb[:, i:i + 1],
                                cs[:, i:i + 1])

                        # o_unnorm^T[(e|Z), t] accumulation (row 64 = Z)
                        for th in range(NTH):
                            tsl = slice(th * THW, (th + 1) * THW)
                            ops = pso.tile([65, THW], FP32, tag="o_ps")
                            for j in range(th * (THW // 128),
                                           (th + 1) * (THW // 128)):
                                jl = slice((j * 128) % THW,
                                           (j * 128) % THW + 128)
                                jc = slice(j * 128, (j + 1) * 128)
                                nc.tensor.matmul(
                                    ops[:, jl], vsb[:, j, h, :], tri_b,
                                    start=True, stop=False)
                                for i in range(j + 1):
                                    nc.tensor.matmul(
                                        ops[:, jl], vsb[:, i, h, :],
                                        esb[:, i, jc],
                                        start=False, stop=(i == j))
                            nc.vector.tensor_add(
                                mu_row[:, tsl], ops[64:65, :], zoff[:, tsl])
                            nc.vector.reciprocal(r_row[:, tsl], mu_row[:, tsl])
                            nc.gpsimd.partition_broadcast(
                                bc[0:64, tsl], r_row[:, tsl])
                            for j in range(th * (THW // 128),
                                           (th + 1) * (THW // 128)):
                                jl = slice((j * 128) % THW,
                                           (j * 128) % THW + 128)
                                jc = slice(j * 128, (j + 1) * 128)
                                nc.vector.scalar_tensor_tensor(
                                    osb[:, h, jc], ops[0:64, jl],
                                    c_sb[0:64, j:j + 1], bc[0:64, jc],
                                    op0=AX.add, op1=AX.mult)
                            nc.sync.dma_start(
                                ag_ins[h // 2][th][:, h % 2, :],
                                osb[:, h, tsl])

                    # ---- AllGather head outputs + full Wo + residual ----
                    # per head-pair h1: delivers ofull subtiles {2r + h1}
                    ofull = bigp.tile([128, KS, T], BF16, tag="big")
                    for h1 in range(2):
                        for th in range(NTH):
                            tsl = slice(th * THW, (th + 1) * THW)
                            if SKIP_COLLECTIVES:
                                nc.sync.dma_start(ag_outs[h1][th][0],
                                                  ag_ins[h1][th][:])
                            else:
                                nc.gpsimd.collective_compute(
                                    "AllGather", AX.bypass,
                                    replica_groups=GROUPS,
                                    ins=[ag_ins[h1][th].opt()],
                                    outs=[ag_outs[h1][th].opt()])
                            # global o-feature f = 256 r + 64 h + e;
                            # partition = 64 h2 + e, subtile = 2 r + h1
                            ag_v = ag_outs[h1][th][:].rearrange(
                                "r e h2 t -> h2 r e t", h2=2)
                            for h2 in range(2):
                                for r_ in range(TP):
                                    nc.sync.dma_start(
                                        ofull[64 * h2:64 * (h2 + 1),
                                              2 * r_ + h1:2 * r_ + h1 + 1,
                                              tsl].rearrange(
                                            "p a t -> p (a t)"),
                                        ag_v[h2, r_])
                    KORD = [2 * r_ for r_ in range(TP)] + \
                           [2 * r_ + 1 for r_ in range(TP)]
                    for th in range(NTH):
                        tc_ = slice(th * THW, (th + 1) * THW)
                        for m in range(KS):
                            ps = ps512.tile([128, THW], FP32, tag="ps512")
                            for ki, k in enumerate(KORD):
                                nc.tensor.matmul(
                                    ps[:], wo[:, k, m * 128:(m + 1) * 128],
                                    ofull[:, k, tc_],
                                    start=(ki == 0), stop=(ki == KS - 1))
                            nc.vector.scalar_tensor_tensor(
                                x_sb[:, m, tc_], ps[:],
                                bo2_ap(m, l, 0), x_sb[:, m, tc_],
                                op0=AX.add, op1=AX.add)

                    # ---- FFN ----
                    hsb = bigp.tile([128, KS, T], BF16, tag="big")
                    for th in range(NTH):
                        tc_ = slice(th * THW, (th + 1) * THW)
                        for m in range(KS):
                            ps = ps512.tile([128, THW], FP32, tag="ps512")
                            for k in range(KS):
                                nc.tensor.matmul(
                                    ps[:],
                                    w1[:, k, m * 128:(m + 1) * 128],
                                    xr(k, tc_),
                                    start=(k == 0), stop=(k == KS - 1))
                            nc.scalar.activation(
                                hsb[:, m, tc_], ps[:],
                                mybir.ActivationFunctionType.Relu,
                                bias=b1_ap(m, l))
                    ffn_ev = bigp.tile([128, KS, T], BF16, tag="big")
                    ar_ret = bigp.tile([128, KS, T], BF16, tag="big")
                    if l == L - 1:
                        xbl = bigp.tile([128, KS, T], BF16, tag="big",
                                        name="xbl")
                    for th in range(NTH):
                        tc_ = slice(th * THW, (th + 1) * THW)
                        arf_in = dp.tile([128, KS, THW], BF16, tag=f"arf_in{th}")
                        arf_out = dp.tile([128, KS, THW], BF16, tag=f"arf_out{th}")
                        for m in range(KS):
                            ps = ps512.tile([128, THW], FP32, tag="ps512")
                            for k in range(KS):
                                nc.tensor.matmul(
                                    ps[:], w2[:, k, m * 128:(m + 1) * 128],
                                    hsb[:, k, tc_],
                                    start=(k == 0), stop=(k == KS - 1))
                            nc.vector.tensor_copy(ffn_ev[:, m, tc_], ps[:])
                            nc.sync.dma_start(
                                arf_in[:, m, :], ffn_ev[:, m, tc_])
                        if SKIP_COLLECTIVES:
                            nc.sync.dma_start(arf_out[:], arf_in[:])
                        else:
                            nc.gpsimd.collective_compute(
                                "AllReduce", AX.add, replica_groups=GROUPS,
                                ins=[arf_in.opt()], outs=[arf_out.opt()])
                        nc.sync.dma_start(ar_ret[:, :, tc_], arf_out[:])
                        for m in range(KS):
                            nc.vector.scalar_tensor_tensor(
                                x_sb[:, m, tc_], ar_ret[:, m, tc_],
                                bo2_ap(m, l, 1), x_sb[:, m, tc_],
                                op0=AX.add, op1=AX.add)
                            if l == L - 1:
                                nc.vector.tensor_copy(
                                    xbl[:, m, tc_], x_sb[:, m, tc_])

            # ---- final LayerNorm (feature-major: stats over partitions) ----
            for th in range(NTH):
                tc_ = slice(th * THW, (th + 1) * THW)
                mp = psmu.tile([1, THW], FP32, tag="mu_ps")
                for k in range(KS):
                    nc.tensor.matmul(
                        mp[:], ones_col, xbl[:, k, tc_],
                        start=(k == 0), stop=(k == KS - 1))
                nc.vector.tensor_scalar(
                    mu_row[:, tc_], mp[:], 1.0 / D, None, op0=AX.mult)
            nc.gpsimd.partition_broadcast(bc[:], mu_row[:])
            ysq = bigp.tile([128, KS, T], BF16, tag="big")
            for k in range(KS):
                nc.vector.tensor_sub(x_sb[:, k, :], x_sb[:, k, :], bc[:])
                nc.vector.tensor_mul(ysq[:, k, :], x_sb[:, k, :], x_sb[:, k, :])
            for th in range(NTH):
                tc_ = slice(th * THW, (th + 1) * THW)
                vp = psmu.tile([1, THW], FP32, tag="mu_ps")
                for k in range(KS):
                    nc.tensor.matmul(
                        vp[:], ones_col, ysq[:, k, tc_],
                        start=(k == 0), stop=(k == KS - 1))
                # s = sqrt(var/D + eps)
                nc.scalar.activation(
                    r_row[:, tc_], vp[:],
                    mybir.ActivationFunctionType.Sqrt,
                    bias=smf[0:1, C_EPS:C_EPS + 1], scale=1.0 / D)
            nc.vector.reciprocal(mu_row[:], r_row[:])   # 1/s
            nc.gpsimd.partition_broadcast(bc[:], mu_row[:])
            xln = bigp.tile([128, KS, T], BF16, tag="big")
            for k in range(KS):
                nc.vector.tensor_mul(x_sb[:, k, :], x_sb[:, k, :], bc[:])
                nc.vector.tensor_scalar(
                    xln[:, k, :], x_sb[:, k, :],
                    smf[:, k:k + 1], smf[:, 8 + k:9 + k],
                    op0=AX.mult, op1=AX.add)

            # ---- logits: x_ln @ Wf + bf  (vocab-sharded) ----
            with tc.tile_pool(name="wf_pool", bufs=4) as wfp, \
                 tc.tile_pool(name="lo_pool", bufs=3) as lop:
                for ch in range(1 if SKIP_LOGITS else VT // VCH):
                    wfc = wfp.tile([128, KS, VCH * 128], BF16, tag="wfc")
                    nc.sync.dma_start(
                        wfc[:],
                        d_wf[:, :, ch * VCH * 128:(ch + 1) * VCH * 128])
                    for vt in range(VCH):
                        m = ch * VCH + vt
                        lo = lop.tile([128, T], FP32, tag="lo")
                        for th in range(NTH):
                            tc_ = slice(th * THW, (th + 1) * THW)
                            ps = ps512.tile([128, THW], FP32, tag="ps512")
                            for k in range(KS):
                                nc.tensor.matmul(
                                    ps[:],
                                    wfc[:, k, vt * 128:(vt + 1) * 128],
                                    xln[:, k, tc_],
                                    start=(k == 0), stop=(k == KS - 1))
                            nc.vector.tensor_scalar(
                                lo[:, tc_], ps[:],
                                smf[:, 16 + m:17 + m], None, op0=AX.add)
                        nc.sync.dma_start(d_out[:, m, :], lo[:])

    nc.compile()
    return nc


def _prep_inputs(idx, tok_emb, pos_emb, Wq, Wk, Wv, Wo, bo, W1, b1, W2, b2,
                 ln_g, ln_b, Wf, bf, T, L):
    """Build the 8 per-core input maps (numpy, host-side sharding)."""
    bf16 = ml_dtypes.bfloat16

    def fsplit(a):
        # [D, ...] -> [128, D//128, ...]: feature d -> (d % 128, d // 128)
        return np.ascontiguousarray(
            a.reshape(a.shape[0] // 128, 128, *a.shape[1:]).swapaxes(0, 1))

    tri = np.triu(np.ones((128, 128), np.float32))  # tri[s,t] = 1 if s<=t

    Wf_pad = np.zeros((D, VPAD), np.float32)
    Wf_pad[:, :V] = Wf
    bf_pad = np.zeros((VPAD,), np.float32)
    bf_pad[:V] = bf

    in_maps = []
    for c in range(8):
        g, r = c // TP, c % TP
        x0 = tok_emb[idx[g, :T]] + pos_emb[:T]          # [T, D]
        xT = np.ascontiguousarray(x0.T)                  # [D, T]

        hsel = [4 * r + h_ for h_ in range(HLOC)]
        # wqk [128, KS, L, 2, 2, 128]
        wqk = np.empty((L, 2, 2, D, 128), np.float32)
        for pair in range(2):
            for qk in range(2):
                Wsrc = Wq if qk == 0 else Wk
                wqk[:, pair, qk, :, 0:64] = Wsrc[:L, hsel[2 * pair]]
                wqk[:, pair, qk, :, 64:128] = Wsrc[:L, hsel[2 * pair + 1]]
        wqk = fsplit(wqk.transpose(3, 0, 1, 2, 4))
        # wv [128, KS, L, 256]
        wv = np.concatenate([Wv[:L, h_] for h_ in hsel], axis=-1)  # [L, D, 256]
        wv = fsplit(wv.transpose(1, 0, 2))
        # wo [128, KS, L, 1024] (full Wo)
        wo = fsplit(Wo[:L].transpose(1, 0, 2))
        # w1 column slice, w2 row slice
        w1 = fsplit(W1[:L, :, FLOC * r:FLOC * (r + 1)].transpose(1, 0, 2))
        w2 = fsplit(W2[:L, FLOC * r:FLOC * (r + 1), :].transpose(1, 0, 2))
        # b1s packed [128, KS*L] with column m*L + l
        b1s = fsplit(b1[:L, FLOC * r:FLOC * (r + 1)].T)            # [128, KS, L]
        b1s = b1s.reshape(128, KS * L)
        # bo2 packed [128, KS*2L] with column m*2L + l*2 + {0,1}
        bo2 = fsplit(np.stack([bo[:L].T, b2[:L].T], axis=-1))      # [128, KS, L, 2]
        bo2 = bo2.reshape(128, KS * L * 2)
        wf = fsplit(Wf_pad[:, VLOC * r:VLOC * (r + 1)])
        bfb = bf_pad[VLOC * r:VLOC * (r + 1)].reshape(VLOC // 128, 128).T
        in_maps.append({
            "x0": fsplit(xT).astype(np.float32),
            "wqk": np.ascontiguousarray(wqk, np.float32),
            "wv": np.ascontiguousarray(wv, np.float32),
            "wo": wo.astype(bf16),
            "w1": np.ascontiguousarray(w1, np.float32),
            "w2": w2.astype(bf16),
            "b1s": np.ascontiguousarray(b1s, np.float32),
            "bo2": np.ascontiguousarray(bo2, np.float32),
            "wf": wf.astype(bf16),
            "bfb": np.ascontiguousarray(bfb, np.float32),
            "lng": np.ascontiguousarray(fsplit(ln_g), np.float32),
            "lnb": np.ascontiguousarray(fsplit(ln_b), np.float32),
            "tri": tri,
        })
    return in_maps


def _assemble(results, T):
    """Per-core lg [128, VT, T] f32 -> logits [B, T, V]."""
    logits = np.empty((B, T, V), np.float32)
    for c in range(8):
        g, r = c // TP, c % TP
        lg = results[c]["lg"].reshape(128, VLOC // 128, T)
        block = lg.transpose(2, 1, 0).reshape(T, VLOC)   # v_loc = 128 m + p
        lo = VLOC * r
        hi = min(VLOC * (r + 1), V)
        if lo < V:
            logits[g, :, lo:hi] = block[:, :hi - lo]
    return logits


_CACHE = {}


def kernel(idx, targets, tok_emb, pos_emb, Wq, Wk, Wv, Wo, bo,
           W1, b1, W2, b2, ln_g, ln_b, Wf, bf):
    T, L = 1024, 6
    f = lambda a: np.asarray(a, np.float32)
    idx = np.asarray(idx)
    targets = np.asarray(targets)

    if "nc" not in _CACHE:
        _CACHE["nc"] = build_nc(T, L)
    nc = _CACHE["nc"]

    in_maps = _prep_inputs(idx, f(tok_emb), f(pos_emb), f(Wq), f(Wk), f(Wv),
                           f(Wo), f(bo), f(W1), f(b1), f(W2), f(b2),
                           f(ln_g), f(ln_b), f(Wf), f(bf), T, L)
    res = run_bass_kernel_spmd(nc, in_maps, core_ids=list(range(8)))
    logits = _assemble(res.results, T)

    # loss on host from the device logits (cheap scalar reduction)
    lg64 = logits.astype(np.float64)
    m = lg64.max(axis=-1, keepdims=True)
    lse = np.log(np.exp(lg64 - m).sum(axis=-1, keepdims=True)) + m
    tgt = np.take_along_axis(lg64, targets.astype(np.int64)[..., None], axis=-1)
    loss = np.float32((lse - tgt).mean())
    return logits, loss
